# revision 6
# baseline (speedup 1.0000x reference)
"""Trainium2 Bass kernel for nn_AttnMech (sparse_attention, no-softmax attention).

Math (reference):
  q/k/v = 2x2-stride-2 convs of pose/app_pose/app  -> [B, 4*64, 48, 48]
  attn  = (Q^T K)/8 (no softmax);  out = attn @ V^T
  out   = gamma_h * out; nearest-upsample 2x; concat with pose; 1x1 conv.

Key algebraic restructure (linear attention => reassociate):
  out_h = V_h (Q_h^T K_h)^T / 8 = (V_h K_h^T) Q_h / 8 = G_h Q_h / 8
with G_h = V_h K_h^T a tiny 64x64 Gram matrix.  The per-head projection,
upsample and final 1x1 conv then fold into:
  final = fw1 @ pose_enc + up2x( W_cat @ Q + fb ) ,
  W_cat[:, 64h:64h+64] = (gamma_h/8) * fw2_h @ G_h
so the huge [2304,2304] attention matrices never exist.

Sharding over 8 cores: core c = (image b = c//2, spatial half = c%2).
Each core convs its half of the image; partial Gram matrices are
AllReduced across the core pair (64 KB); everything else is local.
All matmuls run as float32r (full PE rate for free-dim >= 256).

Layouts:
  - K^T/V^T are produced directly in [pixel, channel] layout (image as the
    stationary matmul operand) so the Gram contraction over pixels needs no
    transposes.  The stationary operand must have a single free dim, so the
    conv input strips are de-interleaved on-chip into the four stride-2
    shifted sub-images (copies spread over DVE/ACT/GPSIMD).
  - Q conv and the full-res pose term use weights as the stationary operand
    and the image as the (multi-dim strided) moving operand.
"""

import os
import sys

for _p in ("/opt/trn_rl_repo", "/root/.axon_site/_ro/trn_rl_repo"):
    if os.path.isdir(_p) and _p not in sys.path:
        sys.path.insert(0, _p)

import numpy as np

import concourse.mybir as mybir
import concourse.tile as tile
from concourse import bacc
from concourse.bass_utils import run_bass_kernel_spmd

F32 = mybir.dt.float32
F32R = mybir.dt.float32r
ADD = mybir.AluOpType.add

P = 128          # partitions
C = 256          # channels
W_IMG = 96       # full-res width
RH = 48          # rows per half (full-res)
FR = RH * W_IMG  # 4608 flat half-image
NI = 24          # local downsampled rows
NJ = 48          # downsampled cols
NLOC = NI * NJ   # 1152 local attn pixels
NSTR = 3         # input strips per K/V conv (16 full-res rows each)
SLEN = 16 * W_IMG    # raw strip length 1536
DLEN = 8 * NJ        # de-interleaved sub-image strip length 384
NMT = 9          # K/V conv m-tiles of 128 (3 per strip)
NT = 3           # Q/z free tiles of 384
QW_T = 384
OT = 12          # output assembly tiles of 384 (4 full-res rows)

_CACHED_NC = None


def _build():
    nc = bacc.Bacc("TRN2", target_bir_lowering=False, debug=False, num_devices=8)

    # per-core inputs (image halves, flattened [256, 4608])
    xq_d = nc.dram_tensor("xq", [C, FR], F32, kind="ExternalInput").ap()
    xk_d = nc.dram_tensor("xk", [C, FR], F32, kind="ExternalInput").ap()
    xv_d = nc.dram_tensor("xv", [C, FR], F32, kind="ExternalInput").ap()
    # conv weights, transposed to [di, dj, in_c, out_c]
    qw_d = nc.dram_tensor("qwT", [2, 2, C, C], F32, kind="ExternalInput").ap()
    kw_d = nc.dram_tensor("kwT", [2, 2, C, C], F32, kind="ExternalInput").ap()
    vw_d = nc.dram_tensor("vwT", [2, 2, C, C], F32, kind="ExternalInput").ap()
    fw1_d = nc.dram_tensor("fw1T", [C, C], F32, kind="ExternalInput").ap()
    fw2_d = nc.dram_tensor("fw2T", [C, C], F32, kind="ExternalInput").ap()
    qb_d = nc.dram_tensor("qb", [C], F32, kind="ExternalInput").ap()
    fb_d = nc.dram_tensor("fb", [C], F32, kind="ExternalInput").ap()
    gsc_d = nc.dram_tensor("gsc", [C], F32, kind="ExternalInput").ap()
    kvb_d = nc.dram_tensor("kvb", [2, C], F32, kind="ExternalInput").ap()
    ones_d = nc.dram_tensor("ones", [1, P], F32, kind="ExternalInput").ap()
    zz_d = nc.dram_tensor("zz", [64, 64], F32, kind="ExternalInput").ap()

    out_d = nc.dram_tensor("out", [C, FR], F32, kind="ExternalOutput").ap()

    # internal DRAM for the Gram-matrix AllReduce
    gpart_d = nc.dram_tensor("g_part", [C, P], F32).ap()
    gred_d = nc.dram_tensor("g_red", [C, P], F32).ap()

    with tile.TileContext(nc) as tc:
        with (
            tc.tile_pool(name="const", bufs=1) as cpool,
            tc.tile_pool(name="img", bufs=1) as ipool,
            tc.tile_pool(name="strip", bufs=2) as spool,
            tc.tile_pool(name="deint", bufs=2) as dpool,
            tc.tile_pool(name="kv", bufs=1) as kvpool,
            tc.tile_pool(name="work", bufs=1) as wpool,
            tc.tile_pool(name="stage", bufs=4) as stpool,
            tc.tile_pool(name="ps_kv", bufs=2, space="PSUM") as pp_kv,
            tc.tile_pool(name="ps_g", bufs=2, space="PSUM") as pp_g,
            tc.tile_pool(name="ps_main", bufs=4, space="PSUM") as pp_main,
        ):
            # ---- constants / weights ----
            qw_sb = cpool.tile([P, 2, 2, 2, C], F32R, tag="qw")
            kw_sb = cpool.tile([P, 2, 2, 2, C], F32R, tag="kw")
            vw_sb = cpool.tile([P, 2, 2, 2, C], F32R, tag="vw")
            for wsb, wd in ((qw_sb, qw_d), (kw_sb, kw_d), (vw_sb, vw_d)):
                for di in range(2):
                    for dj in range(2):
                        nc.sync.dma_start(
                            wsb[:, :, di, dj, :],
                            wd[di, dj].rearrange("(c p) o -> p c o", p=P).bitcast(F32R),
                        )
            fw1_sb = cpool.tile([P, 2, C], F32R, tag="fw1")
            nc.sync.dma_start(
                fw1_sb[:], fw1_d.rearrange("(c p) o -> p c o", p=P).bitcast(F32R)
            )
            fw2_sb = cpool.tile([P, 2, C], F32R, tag="fw2")
            nc.sync.dma_start(
                fw2_sb[:], fw2_d.rearrange("(c p) o -> p c o", p=P).bitcast(F32R)
            )
            qb_sb = cpool.tile([P, 2], F32, tag="qb")
            nc.sync.dma_start(qb_sb[:], qb_d.rearrange("(c p) -> p c", p=P))
            fb_sb = cpool.tile([P, 2], F32, tag="fb")
            nc.sync.dma_start(fb_sb[:], fb_d.rearrange("(c p) -> p c", p=P))
            gsc_sb = cpool.tile([P, 2], F32, tag="gsc")
            nc.sync.dma_start(gsc_sb[:], gsc_d.rearrange("(c p) -> p c", p=P))
            kvb_sb = cpool.tile([1, 2, C], F32R, tag="kvb")
            nc.sync.dma_start(kvb_sb[:], kvb_d[None].bitcast(F32R))
            ones_sb = cpool.tile([1, P], F32R, tag="ones")
            nc.sync.dma_start(ones_sb[:], ones_d.bitcast(F32R))

            # ---- resident pose half (used by Q conv + pose term) ----
            xq_sb = ipool.tile([P, 2, FR], F32R, tag="xq")
            nc.sync.dma_start(
                xq_sb[:], xq_d.rearrange("(c p) f -> p c f", p=P).bitcast(F32R)
            )

            # ---- K^T / V^T convs into [pixel, channel] layout ----
            kt_sb = kvpool.tile([P, NMT, C], F32R, tag="kt")
            vt_sb = kvpool.tile([P, NMT, C], F32R, tag="vt")
            copy_engs = [
                lambda o, i: nc.vector.tensor_copy(o, i),
                lambda o, i: nc.scalar.copy(o, i),
                lambda o, i: nc.gpsimd.tensor_copy(o, i),
            ]

            def conv_t(xd, wsb, kv_idx, dst):
                eng_i = 0
                for s in range(NSTR):
                    raw = spool.tile([P, 2, SLEN], F32R, tag="raw")
                    nc.sync.dma_start(
                        raw[:],
                        xd.rearrange("(c p) f -> p c f", p=P)[
                            :, :, s * SLEN : (s + 1) * SLEN
                        ].bitcast(F32R),
                    )
                    tds = dpool.tile([P, 2, 4, DLEN], F32R, tag="tds")
                    for icc in range(2):
                        rv = raw[:, icc, :].rearrange("p (r w) -> p r w", w=W_IMG)
                        for di in range(2):
                            for dj in range(2):
                                eng = copy_engs[eng_i % 3]
                                eng_i += 1
                                eng(
                                    tds[:, icc, 2 * di + dj, :].rearrange(
                                        "p (i j) -> p i j", j=NJ
                                    ),
                                    rv[:, di::2, dj::2],
                                )
                    for mt in range(3):
                        ps = pp_kv.tile([P, C], F32, tag="pskv")
                        first = True
                        for icc in range(2):
                            for dd in range(4):
                                nc.tensor.matmul(
                                    ps[:],
                                    tds[:, icc, dd, mt * P : (mt + 1) * P],
                                    wsb[:, icc, dd // 2, dd % 2, :],
                                    start=first,
                                    stop=False,
                                )
                                first = False
                        # + bias via rank-1 ones matmul
                        nc.tensor.matmul(
                            ps[:],
                            ones_sb[:],
                            kvb_sb[:, kv_idx, :],
                            start=False,
                            stop=True,
                        )
                        nc.vector.tensor_copy(dst[:, s * 3 + mt, :], ps[:])

            conv_t(xk_d, kw_sb, 0, kt_sb)
            conv_t(xv_d, vw_sb, 1, vt_sb)

            # ---- partial Gram matrices: G[vch, kch] = V^T.T @ K^T ----
            gstage = wpool.tile([P, 2, P], F32, tag="gstage")
            for g in range(2):  # head-pair group: (h0,h1) and (h2,h3)
                psg = pp_g.tile([P, C], F32, tag="psg")
                for t in range(NMT):
                    nc.tensor.matmul(
                        psg[:],
                        vt_sb[:, t, g * P : (g + 1) * P],
                        kt_sb[:, t, :],
                        start=(t == 0),
                        stop=(t == NMT - 1),
                    )
                # keep the same-group column block; scale by gamma_h/8
                nc.vector.tensor_scalar_mul(
                    gstage[:, g, :],
                    psg[:, g * P : (g + 1) * P],
                    gsc_sb[:, g : g + 1],
                )
            nc.sync.dma_start(
                gpart_d.rearrange("(g p) k -> p g k", p=P), gstage[:]
            )
            nc.gpsimd.collective_compute(
                "AllReduce",
                ADD,
                replica_groups=[[0, 1], [2, 3], [4, 5], [6, 7]],
                ins=[gpart_d],
                outs=[gred_d],
            )
            # load back per-head diagonal 64x64 blocks into block-diag tiles
            g_sb = wpool.tile([P, 2, P], F32R, tag="gsb")
            for g in range(2):
                for hh in range(2):
                    r0 = 64 * hh
                    nc.sync.dma_start(
                        g_sb[r0 : r0 + 64, g, r0 : r0 + 64],
                        gred_d[g * P + r0 : g * P + r0 + 64, r0 : r0 + 64].bitcast(
                            F32R
                        ),
                    )
                    # zero the off-diagonal cross-head blocks
                    r1 = 64 - r0
                    nc.sync.dma_start(
                        g_sb[r0 : r0 + 64, g, r1 : r1 + 64], zz_d.bitcast(F32R)
                    )

            # ---- Q conv (standard [channel, n] layout) ----
            q_sb = wpool.tile([P, 2, NLOC], F32R, tag="q")
            xqv = [
                xq_sb[:, icc, :].rearrange("p (r w) -> p r w", w=W_IMG)
                for icc in range(2)
            ]
            for qcc in range(2):
                for nt in range(NT):
                    ps = pp_main.tile([P, QW_T], F32, tag="psmain")
                    psv = ps[:].rearrange("p (i j) -> p i j", j=NJ)
                    first = True
                    for icc in range(2):
                        for di in range(2):
                            for dj in range(2):
                                nc.tensor.matmul(
                                    psv,
                                    qw_sb[:, icc, di, dj, qcc * P : (qcc + 1) * P],
                                    xqv[icc][
                                        :, 16 * nt + di : 16 * nt + 16 : 2, dj::2
                                    ],
                                    start=first,
                                    stop=(icc == 1 and di == 1 and dj == 1),
                                )
                                first = False
                    nc.vector.tensor_tensor(
                        q_sb[:, qcc, nt * QW_T : (nt + 1) * QW_T],
                        ps[:],
                        qb_sb[:, qcc : qcc + 1].to_broadcast([P, QW_T]),
                        ADD,
                    )

            # ---- W_cat^T = blockdiag(G) @ fw2^T  (per group) ----
            w_sb = wpool.tile([P, 2, C], F32R, tag="w")
            for g in range(2):
                psw = pp_g.tile([P, C], F32, tag="psg")
                nc.tensor.matmul(
                    psw[:], g_sb[:, g, :], fw2_sb[:, g, :], start=True, stop=True
                )
                nc.vector.tensor_copy(w_sb[:, g, :], psw[:])

            # ---- z'' = W_cat^T.T @ Q + fb  (downsampled grid) ----
            z_sb = wpool.tile([P, 2, NLOC], F32, tag="z")
            for oc in range(2):
                for nt in range(NT):
                    ps = pp_main.tile([P, QW_T], F32, tag="psmain")
                    for g in range(2):
                        nc.tensor.matmul(
                            ps[:],
                            w_sb[:, g, oc * P : (oc + 1) * P],
                            q_sb[:, g, nt * QW_T : (nt + 1) * QW_T],
                            start=(g == 0),
                            stop=(g == 1),
                        )
                    nc.vector.tensor_tensor(
                        z_sb[:, oc, nt * QW_T : (nt + 1) * QW_T],
                        ps[:],
                        fb_sb[:, oc : oc + 1].to_broadcast([P, QW_T]),
                        ADD,
                    )

            # ---- pose term + fused 2x upsample-add of z'' ----
            outv = out_d.rearrange("(c p) f -> p c f", p=P)
            for oc in range(2):
                zv = z_sb[:, oc, :].rearrange("p (i j) -> p i j", j=NJ)
                for ot in range(OT):
                    ps = pp_main.tile([P, QW_T], F32, tag="psmain")
                    for icc in range(2):
                        nc.tensor.matmul(
                            ps[:],
                            fw1_sb[:, icc, oc * P : (oc + 1) * P],
                            xq_sb[:, icc, ot * QW_T : (ot + 1) * QW_T],
                            start=(icc == 0),
                            stop=(icc == 1),
                        )
                    st = stpool.tile([P, QW_T], F32, tag="ostage")
                    psv = ps[:].rearrange(
                        "p (i ri j rj) -> p i ri j rj", i=2, ri=2, j=NJ, rj=2
                    )
                    stv = st[:].rearrange(
                        "p (i ri j rj) -> p i ri j rj", i=2, ri=2, j=NJ, rj=2
                    )
                    zb = zv[:, 2 * ot : 2 * ot + 2, :, None].to_broadcast(
                        [P, 2, NJ, 2]
                    )
                    for ri in range(2):
                        nc.vector.tensor_tensor(
                            stv[:, :, ri, :, :], psv[:, :, ri, :, :], zb, ADD
                        )
                    nc.sync.dma_start(
                        outv[:, oc, ot * QW_T : (ot + 1) * QW_T], st[:]
                    )

    nc.compile()
    return nc


def _prep_inputs(app_enc, app_pose_enc, pose_enc, qw, qb, kw, kb, vb, vw, gamma, fw, fb):
    """Build the 8 per-core input maps (host-side shard + weight transforms)."""
    f = np.float32
    qwT = np.ascontiguousarray(qw.transpose(2, 3, 1, 0), dtype=f)
    kwT = np.ascontiguousarray(kw.transpose(2, 3, 1, 0), dtype=f)
    vwT = np.ascontiguousarray(vw.transpose(2, 3, 1, 0), dtype=f)
    fw1T = np.ascontiguousarray(fw[:, :C, 0, 0].T, dtype=f)
    fw2T = np.ascontiguousarray(fw[:, C:, 0, 0].T, dtype=f)
    gsc = (np.repeat(gamma.astype(np.float64), 64) / 8.0).astype(f)
    kvb = np.stack([kb, vb]).astype(f)
    ones = np.ones((1, P), dtype=f)
    zz = np.zeros((64, 64), dtype=f)
    common = {
        "qwT": qwT, "kwT": kwT, "vwT": vwT,
        "fw1T": fw1T, "fw2T": fw2T,
        "qb": qb.astype(f), "fb": fb.astype(f),
        "gsc": gsc, "kvb": kvb, "ones": ones, "zz": zz,
    }
    in_maps = []
    for c in range(8):
        b, h = c // 2, c % 2
        rows = slice(RH * h, RH * (h + 1))
        in_maps.append({
            "xq": np.ascontiguousarray(pose_enc[b, :, rows, :], dtype=f).reshape(C, FR),
            "xk": np.ascontiguousarray(app_pose_enc[b, :, rows, :], dtype=f).reshape(C, FR),
            "xv": np.ascontiguousarray(app_enc[b, :, rows, :], dtype=f).reshape(C, FR),
            **common,
        })
    return in_maps


def _run(inputs, trace=False):
    global _CACHED_NC
    if _CACHED_NC is None:
        _CACHED_NC = _build()
    nc = _CACHED_NC
    inputs = {k: np.asarray(v) for k, v in inputs.items()}
    in_maps = _prep_inputs(
        inputs["app_enc"], inputs["app_pose_enc"], inputs["pose_enc"],
        inputs["qw"], inputs["qb"], inputs["kw"], inputs["kb"],
        inputs["vb"], inputs["vw"], inputs["gamma"], inputs["fw"], inputs["fb"],
    )
    res = run_bass_kernel_spmd(nc, in_maps, list(range(8)), trace=trace)
    out = np.empty((4, C, W_IMG, W_IMG), dtype=np.float32)
    for c in range(8):
        b, h = c // 2, c % 2
        out[b, :, RH * h : RH * (h + 1), :] = res.results[c]["out"].reshape(C, RH, W_IMG)
    return out, res


def kernel(**inputs):
    out, _ = _run(inputs, trace=False)
    return out


# revision 9
# speedup vs baseline: 1.0545x; 1.0545x over previous
"""Trainium2 Bass kernel for nn_AttnMech (sparse_attention, no-softmax attention).

Math (reference):
  q/k/v = 2x2-stride-2 convs of pose/app_pose/app  -> [B, 4*64, 48, 48]
  attn  = (Q^T K)/8 (no softmax);  out = attn @ V^T
  out   = gamma_h * out; nearest-upsample 2x; concat with pose; 1x1 conv.

Key algebraic restructure (linear attention => reassociate):
  out_h = V_h (Q_h^T K_h)^T / 8 = (V_h K_h^T) Q_h / 8 = G_h Q_h / 8
with G_h = V_h K_h^T a tiny 64x64 Gram matrix.  The per-head projection,
upsample and final 1x1 conv then fold into:
  final = fw1 @ pose_enc + up2x( W_cat @ Q + fb ) ,
  W_cat[:, 64h:64h+64] = (gamma_h/8) * fw2_h @ G_h
so the huge [2304,2304] attention matrices never exist.

Sharding over 8 cores: core c = (image b = c//2, spatial half = c%2).
Each core convs its half of the image; partial Gram matrices are
AllReduced across the core pair (64 KB); everything else is local.
All matmuls run as float32r (full PE rate for free-dim >= 256).

Implementation notes:
  - All convs keep weights as the stationary operand (single free dim as
    the hardware requires) and stream the image with multi-dim strided
    moving APs.  K/V results are then PE-transposed per 128-pixel chunk
    and immediately consumed by the Gram accumulation.
  - All constants ship in one packed [128, WLEN] DRAM blob (one DMA,
    one descriptor run per partition); image tensors are laid out
    host-side so every DMA is one contiguous run per partition.
  - The Q conv is scheduled after the AllReduce launch so the PE has
    work while the collective's ~20us fixed latency elapses.
"""

import os
import sys

for _p in ("/opt/trn_rl_repo", "/root/.axon_site/_ro/trn_rl_repo"):
    if os.path.isdir(_p) and _p not in sys.path:
        sys.path.insert(0, _p)

import numpy as np

import concourse.mybir as mybir
import concourse.tile as tile
from concourse import bacc
from concourse.bass_utils import run_bass_kernel_spmd

F32 = mybir.dt.float32
F32R = mybir.dt.float32r
ADD = mybir.AluOpType.add
IDENT = mybir.ActivationFunctionType.Identity

P = 128          # partitions
C = 256          # channels
W_IMG = 96       # full-res width
RH = 48          # rows per half (full-res)
FR = RH * W_IMG  # 4608 flat half-image
NI = 24          # local downsampled rows
NJ = 48          # downsampled cols
NLOC = NI * NJ   # 1152 local attn pixels
NT = 3           # conv free tiles of 384 (16 full-res rows each)
TW = 384
SLEN = 2 * 16 * W_IMG  # strip len per partition (both ic chunks) 3072
NMT = 9          # 128-pixel chunks of the local grid
OT = 12          # output assembly tiles of 384 (4 full-res rows)

# wpack layout (per partition, fp32 words)
QW_O, KW_O, VW_O = 0, 2048, 4096
FW1_O, FW2_O = 6144, 6656
ID_O = 7168
QB_O, FB_O, GSC_O, KB_O, VB_O = 7296, 7298, 7300, 7302, 7304
WLEN = 7306

_CACHED_NC = None


def _build():
    nc = bacc.Bacc("TRN2", target_bir_lowering=False, debug=False, num_devices=8)

    xq_d = nc.dram_tensor("xq", [P, 2, FR], F32, kind="ExternalInput").ap()
    xk_d = nc.dram_tensor("xk", [P, NT, SLEN], F32, kind="ExternalInput").ap()
    xv_d = nc.dram_tensor("xv", [P, NT, SLEN], F32, kind="ExternalInput").ap()
    wpack_d = nc.dram_tensor("wpack", [P, WLEN], F32, kind="ExternalInput").ap()
    zz_d = nc.dram_tensor("zz", [64, 64], F32, kind="ExternalInput").ap()

    out_d = nc.dram_tensor("out", [P, 2, FR], F32, kind="ExternalOutput").ap()

    gpart_d = nc.dram_tensor("g_part", [P, C], F32).ap()
    gred_d = nc.dram_tensor("g_red", [P, C], F32).ap()

    with tile.TileContext(nc) as tc:
        with (
            tc.tile_pool(name="const", bufs=1) as cpool,
            tc.tile_pool(name="img", bufs=1) as ipool,
            tc.tile_pool(name="mid", bufs=2) as mpool,
            tc.tile_pool(name="kvt", bufs=4) as tpool,
            tc.tile_pool(name="work", bufs=1) as wpool,
            tc.tile_pool(name="ps", bufs=8, space="PSUM") as psp,
        ):
            # ---- constants (one DMA) ----
            wp = cpool.tile([P, WLEN], F32R, tag="wp")
            nc.sync.dma_start(wp[:], wpack_d.bitcast(F32R))
            qw_v = wp[:, QW_O : QW_O + 2048].rearrange(
                "p (i d o) -> p i d o", i=2, d=4
            )
            kw_v = wp[:, KW_O : KW_O + 2048].rearrange(
                "p (i d o) -> p i d o", i=2, d=4
            )
            vw_v = wp[:, VW_O : VW_O + 2048].rearrange(
                "p (i d o) -> p i d o", i=2, d=4
            )
            fw1_v = wp[:, FW1_O : FW1_O + 512].rearrange("p (i o) -> p i o", i=2)
            fw2_v = wp[:, FW2_O : FW2_O + 512].rearrange("p (i o) -> p i o", i=2)
            id_v = wp[:, ID_O : ID_O + P]

            def sca(off):  # [P, 1] fp32 per-partition scalar view
                return wp[:, off : off + 2].bitcast(F32)

            # ---- images ----
            xk_sb = ipool.tile([P, NT, SLEN], F32R, tag="xk")
            xv_sb = ipool.tile([P, NT, SLEN], F32R, tag="xv")
            for s in range(NT):
                nc.sync.dma_start(xk_sb[:, s], xk_d[:, s].bitcast(F32R))
            for s in range(NT):
                nc.sync.dma_start(xv_sb[:, s], xv_d[:, s].bitcast(F32R))
            xq_sb = ipool.tile([P, 2, FR], F32R, tag="xq")
            nc.sync.dma_start(xq_sb[:], xq_d.bitcast(F32R))

            # ---- K / V convs, channel-major [ch, m] ----
            k_sb = mpool.tile([P, 2, NLOC], F32R, tag="mid")
            v_sb = mpool.tile([P, 2, NLOC], F32R, tag="mid")

            def conv_cm(src_sb, w_v, bias_off, dst, flip):
                # src_sb [P, NT, 2, 16*W]; strip s == free tile s
                for occ in range(2):
                    for s in range(NT):
                        ps = psp.tile([P, TW], F32, tag="ps")
                        psv = ps[:].rearrange("p (i j) -> p i j", j=NJ)
                        first = True
                        for icc in range(2):
                            sv = src_sb[:, s, icc, :].rearrange(
                                "p (r w) -> p r w", w=W_IMG
                            )
                            for dd in range(4):
                                di, dj = dd // 2, dd % 2
                                nc.tensor.matmul(
                                    psv,
                                    w_v[:, icc, dd, occ * P : (occ + 1) * P],
                                    sv[:, di::2, dj::2],
                                    start=first,
                                    stop=(icc == 1 and dd == 3),
                                )
                                first = False
                        dslice = dst[:, occ, s * TW : (s + 1) * TW]
                        if flip:
                            nc.scalar.activation(
                                dslice, ps[:], IDENT,
                                bias=sca(bias_off)[:, occ : occ + 1], scale=1.0,
                            )
                        else:
                            nc.vector.tensor_tensor(
                                dslice, ps[:],
                                sca(bias_off)[:, occ : occ + 1].to_broadcast([P, TW]),
                                ADD,
                            )

            conv_cm(
                xk_sb[:].rearrange("p s (i f) -> p s i f", i=2), kw_v, KB_O, k_sb, False
            )
            conv_cm(
                xv_sb[:].rearrange("p s (i f) -> p s i f", i=2), vw_v, VB_O, v_sb, True
            )

            # ---- per-chunk transpose + Gram accumulation ----
            gps = [
                psp.tile([P, C], F32, tag="ps", name=f"gps{g}") for g in range(2)
            ]
            flip = 0
            for t in range(NMT):
                ktt = tpool.tile([P, C], F32R, tag="ktt")
                vtt = tpool.tile([P, C], F32R, tag="vtt")
                for src, dst in ((k_sb, ktt), (v_sb, vtt)):
                    for occ in range(2):
                        tp = psp.tile([P, P], F32R, tag="ps")
                        nc.tensor.transpose(
                            tp[:], src[:, occ, t * P : (t + 1) * P], id_v
                        )
                        dsl = dst[:, occ * P : (occ + 1) * P]
                        if flip % 2:
                            nc.scalar.copy(dsl, tp[:])
                        else:
                            nc.vector.tensor_copy(dsl, tp[:])
                        flip += 1
                for g in range(2):
                    nc.tensor.matmul(
                        gps[g][:],
                        vtt[:, g * P : (g + 1) * P],
                        ktt[:],
                        start=(t == 0),
                        stop=(t == NMT - 1),
                        skip_group_check=True,
                    )

            gstage = wpool.tile([P, 2, P], F32, tag="gstage")
            for g in range(2):
                nc.vector.tensor_scalar_mul(
                    gstage[:, g, :],
                    gps[g][:, g * P : (g + 1) * P],
                    sca(GSC_O)[:, g : g + 1],
                )
            nc.sync.dma_start(gpart_d, gstage[:])
            nc.gpsimd.collective_compute(
                "AllReduce",
                ADD,
                replica_groups=[[0, 1], [2, 3], [4, 5], [6, 7]],
                ins=[gpart_d],
                outs=[gred_d],
            )
            g_sb = wpool.tile([P, 2, P], F32R, tag="gsb")
            for g in range(2):
                for hh in range(2):
                    r0 = 64 * hh
                    r1 = 64 - r0
                    nc.sync.dma_start(
                        g_sb[r0 : r0 + 64, g, r0 : r0 + 64],
                        gred_d[
                            r0 : r0 + 64, g * P + r0 : g * P + r0 + 64
                        ].bitcast(F32R),
                    )
                    nc.sync.dma_start(
                        g_sb[r0 : r0 + 64, g, r1 : r1 + 64], zz_d.bitcast(F32R)
                    )

            # ---- Q conv (fills the collective's latency) ----
            q_sb = wpool.tile([P, 2, NLOC], F32R, tag="q")
            xqv = [
                xq_sb[:, icc, :].rearrange("p (r w) -> p r w", w=W_IMG)
                for icc in range(2)
            ]
            for qcc in range(2):
                for nt in range(NT):
                    ps = psp.tile([P, TW], F32, tag="ps")
                    psv = ps[:].rearrange("p (i j) -> p i j", j=NJ)
                    first = True
                    for icc in range(2):
                        for dd in range(4):
                            di, dj = dd // 2, dd % 2
                            nc.tensor.matmul(
                                psv,
                                qw_v[:, icc, dd, qcc * P : (qcc + 1) * P],
                                xqv[icc][:, 16 * nt + di : 16 * nt + 16 : 2, dj::2],
                                start=first,
                                stop=(icc == 1 and dd == 3),
                            )
                            first = False
                    if nt % 2:
                        nc.scalar.activation(
                            q_sb[:, qcc, nt * TW : (nt + 1) * TW], ps[:], IDENT,
                            bias=sca(QB_O)[:, qcc : qcc + 1], scale=1.0,
                        )
                    else:
                        nc.vector.tensor_tensor(
                            q_sb[:, qcc, nt * TW : (nt + 1) * TW], ps[:],
                            sca(QB_O)[:, qcc : qcc + 1].to_broadcast([P, TW]),
                            ADD,
                        )

            # ---- W_cat^T = blockdiag(G) @ fw2^T ----
            w_sb = wpool.tile([P, 2, C], F32R, tag="w")
            for g in range(2):
                psw = psp.tile([P, C], F32, tag="ps")
                nc.tensor.matmul(
                    psw[:], g_sb[:, g, :], fw2_v[:, g, :], start=True, stop=True
                )
                nc.vector.tensor_copy(w_sb[:, g, :], psw[:])

            # ---- z'' = W_cat^T.T @ Q + fb ----
            z_sb = wpool.tile([P, 2, NLOC], F32, tag="z")
            for oc in range(2):
                for nt in range(NT):
                    ps = psp.tile([P, TW], F32, tag="ps")
                    for g in range(2):
                        nc.tensor.matmul(
                            ps[:],
                            w_sb[:, g, oc * P : (oc + 1) * P],
                            q_sb[:, g, nt * TW : (nt + 1) * TW],
                            start=(g == 0),
                            stop=(g == 1),
                        )
                    if nt % 2:
                        nc.scalar.activation(
                            z_sb[:, oc, nt * TW : (nt + 1) * TW], ps[:], IDENT,
                            bias=sca(FB_O)[:, oc : oc + 1], scale=1.0,
                        )
                    else:
                        nc.vector.tensor_tensor(
                            z_sb[:, oc, nt * TW : (nt + 1) * TW], ps[:],
                            sca(FB_O)[:, oc : oc + 1].to_broadcast([P, TW]),
                            ADD,
                        )

            # ---- pose term + fused 2x upsample-add of z'' ----
            for oc in range(2):
                zv = z_sb[:, oc, :].rearrange("p (i j) -> p i j", j=NJ)
                for half in range(2):
                    ost = mpool.tile([P, 6 * TW], F32, tag="mid")
                    for k in range(6):
                        ot = half * 6 + k
                        ps = psp.tile([P, TW], F32, tag="ps")
                        for icc in range(2):
                            nc.tensor.matmul(
                                ps[:],
                                fw1_v[:, icc, oc * P : (oc + 1) * P],
                                xq_sb[:, icc, ot * TW : (ot + 1) * TW],
                                start=(icc == 0),
                                stop=(icc == 1),
                            )
                        psv = ps[:].rearrange(
                            "p (i ri j rj) -> p i ri j rj", i=2, ri=2, j=NJ, rj=2
                        )
                        stv = ost[:, k * TW : (k + 1) * TW].rearrange(
                            "p (i ri j rj) -> p i ri j rj", i=2, ri=2, j=NJ, rj=2
                        )
                        zb = zv[:, 2 * ot : 2 * ot + 2, :, None].to_broadcast(
                            [P, 2, NJ, 2]
                        )
                        for ri in range(2):
                            nc.vector.tensor_tensor(
                                stv[:, :, ri, :, :], psv[:, :, ri, :, :], zb, ADD
                            )
                    nc.sync.dma_start(
                        out_d[:, oc, half * 6 * TW : (half + 1) * 6 * TW], ost[:]
                    )

    nc.compile()
    return nc


def _prep_inputs(inputs):
    """Build the 8 per-core input maps (host-side shard + weight packing)."""
    f = np.float32
    qw, qb = np.asarray(inputs["qw"], f), np.asarray(inputs["qb"], f)
    kw, kb = np.asarray(inputs["kw"], f), np.asarray(inputs["kb"], f)
    vw, vb = np.asarray(inputs["vw"], f), np.asarray(inputs["vb"], f)
    gamma = np.asarray(inputs["gamma"], f)
    fw, fb = np.asarray(inputs["fw"], f), np.asarray(inputs["fb"], f)
    pose = np.asarray(inputs["pose_enc"], f)
    app_pose = np.asarray(inputs["app_pose_enc"], f)
    app = np.asarray(inputs["app_enc"], f)

    wpack = np.zeros((P, WLEN), dtype=f)

    def packw(dst_off, w):
        # w [oc, ic, 2, 2] -> [p, icc, dd, oc]
        t = w.transpose(1, 2, 3, 0).reshape(2, P, 4, C).transpose(1, 0, 2, 3)
        wpack[:, dst_off : dst_off + 2048] = t.reshape(P, 2048)

    packw(QW_O, qw)
    packw(KW_O, kw)
    packw(VW_O, vw)
    wpack[:, FW1_O : FW1_O + 512] = (
        fw[:, :C, 0, 0].T.reshape(2, P, C).transpose(1, 0, 2).reshape(P, 512)
    )
    wpack[:, FW2_O : FW2_O + 512] = (
        fw[:, C:, 0, 0].T.reshape(2, P, C).transpose(1, 0, 2).reshape(P, 512)
    )
    wpack[:, ID_O : ID_O + P] = np.eye(P, dtype=f)
    wpack[:, QB_O : QB_O + 2] = qb.reshape(2, P).T
    wpack[:, FB_O : FB_O + 2] = fb.reshape(2, P).T
    gsc = (np.repeat(gamma.astype(np.float64), 64) / 8.0).astype(f)
    wpack[:, GSC_O : GSC_O + 2] = gsc.reshape(2, P).T
    wpack[:, KB_O : KB_O + 2] = kb.reshape(2, P).T
    wpack[:, VB_O : VB_O + 2] = vb.reshape(2, P).T
    zz = np.zeros((64, 64), dtype=f)

    def shard_q(x, b, h):  # [p, icc, fr]
        halfimg = x[b, :, RH * h : RH * (h + 1), :].reshape(2, P, FR)
        return np.ascontiguousarray(halfimg.transpose(1, 0, 2))

    def shard_kv(x, b, h):  # [p, strip, icc*1536]
        halfimg = x[b, :, RH * h : RH * (h + 1), :].reshape(2, P, NT, SLEN // 2)
        return np.ascontiguousarray(halfimg.transpose(1, 2, 0, 3).reshape(P, NT, SLEN))

    in_maps = []
    for c in range(8):
        b, h = c // 2, c % 2
        in_maps.append({
            "xq": shard_q(pose, b, h),
            "xk": shard_kv(app_pose, b, h),
            "xv": shard_kv(app, b, h),
            "wpack": wpack,
            "zz": zz,
        })
    return in_maps


def _run(inputs, trace=False):
    global _CACHED_NC
    if _CACHED_NC is None:
        _CACHED_NC = _build()
    nc = _CACHED_NC
    in_maps = _prep_inputs(inputs)
    res = run_bass_kernel_spmd(nc, in_maps, list(range(8)), trace=trace)
    out = np.empty((4, C, W_IMG, W_IMG), dtype=np.float32)
    for c in range(8):
        b, h = c // 2, c % 2
        o = res.results[c]["out"]  # [P, 2, FR]
        out[b, :, RH * h : RH * (h + 1), :] = o.transpose(1, 0, 2).reshape(
            C, RH, W_IMG
        )
    return out, res


def kernel(**inputs):
    out, _ = _run(inputs, trace=False)
    return out


# revision 15
# speedup vs baseline: 1.0979x; 1.0411x over previous
"""Trainium2 Bass kernel for nn_AttnMech (sparse_attention, no-softmax attention).

Math (reference):
  q/k/v = 2x2-stride-2 convs of pose/app_pose/app  -> [B, 4*64, 48, 48]
  attn  = (Q^T K)/8 (no softmax);  out = attn @ V^T
  out   = gamma_h * out; nearest-upsample 2x; concat with pose; 1x1 conv.

Key algebraic restructure (linear attention => reassociate):
  out_h = V_h (Q_h^T K_h)^T / 8 = (V_h K_h^T) Q_h / 8 = G_h Q_h / 8
with G_h = V_h K_h^T a tiny 64x64 Gram matrix.  The per-head projection,
upsample and final 1x1 conv then fold into:
  final = fw1 @ pose_enc + up2x( W_cat @ Q + fb ) ,
  W_cat[:, 64h:64h+64] = (gamma_h/8) * fw2_h @ G_h
so the huge [2304,2304] attention matrices never exist.

Sharding over 8 cores: core c = (image b = c//2, spatial half = c%2).
Each core convs its half of the image; partial Gram matrices are
AllReduced across the core pair (64 KB); everything else is local.
All matmuls run as float32r (full PE rate for free-dim >= 256).

Implementation notes:
  - All convs keep weights as the stationary operand (single free dim as
    the hardware requires) and stream the image with multi-dim strided
    moving APs.  K/V results are then PE-transposed per 128-pixel chunk
    and immediately consumed by the Gram accumulation.
  - All constants ship in one packed [128, WLEN] DRAM blob (one DMA,
    one descriptor run per partition); image tensors are laid out
    host-side so every DMA is one contiguous run per partition.
  - The Q conv is scheduled after the AllReduce launch so the PE has
    work while the collective's ~20us fixed latency elapses.
"""

import os
import sys

for _p in ("/opt/trn_rl_repo", "/root/.axon_site/_ro/trn_rl_repo"):
    if os.path.isdir(_p) and _p not in sys.path:
        sys.path.insert(0, _p)

import numpy as np

import concourse.mybir as mybir
import concourse.tile as tile
from concourse import bacc, bass2jax
from concourse.bass_utils import run_bass_kernel_spmd

F32 = mybir.dt.float32
F32R = mybir.dt.float32r
ADD = mybir.AluOpType.add
IDENT = mybir.ActivationFunctionType.Identity

P = 128          # partitions
C = 256          # channels
W_IMG = 96       # full-res width
RH = 48          # rows per half (full-res)
FR = RH * W_IMG  # 4608 flat half-image
NI = 24          # local downsampled rows
NJ = 48          # downsampled cols
NLOC = NI * NJ   # 1152 local attn pixels
NT = 3           # conv free tiles of 384 (16 full-res rows each)
TW = 384
SLEN = 2 * 16 * W_IMG  # strip len per partition (both ic chunks) 3072
NMT = 9          # 128-pixel chunks of the local grid
OT = 12          # output assembly tiles of 384 (4 full-res rows)

# wpack layout (per partition, fp32 words)
QW_O, KW_O, VW_O = 0, 2048, 4096
FW1_O, FW2_O = 6144, 6656
ID_O = 7168
QB_O, FB_O, GSC_O, KB_O, VB_O = 7296, 7298, 7300, 7302, 7304
WLEN = 7306

_CACHED_NC = None
_RUNNER = None


def _make_runner(nc, n_cores=8):
    """Like bass2jax.run_bass_via_pjrt, but inputs are pre-placed on the
    devices (parallel transfer + aligned core start) and the jitted
    executable is cached across calls."""
    import jax
    from jax.experimental.shard_map import shard_map
    from jax.sharding import Mesh, NamedSharding, PartitionSpec

    bass2jax.install_neuronx_cc_hook()

    partition_name = (
        nc.partition_id_tensor.name if nc.partition_id_tensor else None
    )
    in_names, out_names, out_avals = [], [], []
    for alloc in nc.m.functions[0].allocations:
        if not isinstance(alloc, mybir.MemoryLocationSet):
            continue
        name = alloc.memorylocations[0].name
        if alloc.kind == "ExternalInput":
            if name != partition_name:
                in_names.append(name)
        elif alloc.kind == "ExternalOutput":
            out_avals.append(
                jax.core.ShapedArray(
                    tuple(alloc.tensor_shape), mybir.dt.np(alloc.dtype)
                )
            )
            out_names.append(name)
    n_params = len(in_names)
    all_in = tuple(in_names + out_names)
    if partition_name is not None:
        all_in = all_in + (partition_name,)

    def _body(*args):
        operands = list(args)
        if partition_name is not None:
            operands.append(bass2jax.partition_id_tensor())
        return tuple(
            bass2jax._bass_exec_p.bind(
                *operands,
                out_avals=tuple(out_avals),
                in_names=all_in,
                out_names=tuple(out_names),
                lowering_input_output_aliases=(),
                sim_require_finite=True,
                sim_require_nnan=True,
                nc=nc,
            )
        )

    devices = jax.devices()[:n_cores]
    mesh = Mesh(np.asarray(devices), ("core",))
    nspec = n_params + len(out_names)
    donate = tuple(range(n_params, nspec))
    sharded = jax.jit(
        shard_map(
            _body,
            mesh=mesh,
            in_specs=(PartitionSpec("core"),) * nspec,
            out_specs=(PartitionSpec("core"),) * len(out_names),
            check_rep=False,
        ),
        donate_argnums=donate,
        keep_unused=True,
    )
    sh = NamedSharding(mesh, PartitionSpec("core"))

    def run(in_maps):
        concat_in = [
            jax.device_put(
                np.concatenate([np.asarray(m[nm]) for m in in_maps], axis=0), sh
            )
            for nm in in_names
        ]
        concat_zeros = [
            jax.device_put(
                np.zeros((n_cores * a.shape[0], *a.shape[1:]), a.dtype), sh
            )
            for a in out_avals
        ]
        jax.block_until_ready(concat_in)
        jax.block_until_ready(concat_zeros)
        out_arrs = sharded(*concat_in, *concat_zeros)
        jax.block_until_ready(out_arrs)
        return [
            {
                nm: np.asarray(out_arrs[i]).reshape(n_cores, *out_avals[i].shape)[c]
                for i, nm in enumerate(out_names)
            }
            for c in range(n_cores)
        ]

    return run


def _build():
    nc = bacc.Bacc("TRN2", target_bir_lowering=False, debug=False, num_devices=8)

    xq_d = nc.dram_tensor("xq", [P, 2, FR], F32, kind="ExternalInput").ap()
    xk_d = nc.dram_tensor("xk", [P, NT, SLEN], F32, kind="ExternalInput").ap()
    xv_d = nc.dram_tensor("xv", [P, NT, SLEN], F32, kind="ExternalInput").ap()
    wpack_d = nc.dram_tensor("wpack", [P, WLEN], F32, kind="ExternalInput").ap()
    zz_d = nc.dram_tensor("zz", [64, 64], F32, kind="ExternalInput").ap()

    out_d = nc.dram_tensor("out", [P, 2, FR], F32, kind="ExternalOutput").ap()

    gpart_d = nc.dram_tensor("g_part", [P, C], F32).ap()
    gred_d = nc.dram_tensor("g_red", [P, C], F32).ap()

    with tile.TileContext(nc) as tc:
        with (
            tc.tile_pool(name="const", bufs=1) as cpool,
            tc.tile_pool(name="img", bufs=1) as ipool,
            tc.tile_pool(name="mid", bufs=2) as mpool,
            tc.tile_pool(name="kvt", bufs=4) as tpool,
            tc.tile_pool(name="work", bufs=1) as wpool,
            tc.tile_pool(name="ps", bufs=8, space="PSUM") as psp,
        ):
            # ---- constants, split so the K-conv prerequisites land first ----
            wp = cpool.tile([P, WLEN], F32R, tag="wp")
            nc.sync.dma_start(
                wp[:, KW_O:], wpack_d[:, KW_O:].bitcast(F32R)
            )  # kw, vw, fw, identity, biases
            qw_v = wp[:, QW_O : QW_O + 2048].rearrange(
                "p (i d o) -> p i d o", i=2, d=4
            )
            kw_v = wp[:, KW_O : KW_O + 2048].rearrange(
                "p (i d o) -> p i d o", i=2, d=4
            )
            vw_v = wp[:, VW_O : VW_O + 2048].rearrange(
                "p (i d o) -> p i d o", i=2, d=4
            )
            fw1_v = wp[:, FW1_O : FW1_O + 512].rearrange("p (i o) -> p i o", i=2)
            fw2_v = wp[:, FW2_O : FW2_O + 512].rearrange("p (i o) -> p i o", i=2)
            id_v = wp[:, ID_O : ID_O + P]

            def sca(off):  # [P, 1] fp32 per-partition scalar view
                return wp[:, off : off + 2].bitcast(F32)

            # ---- images ----
            xk_sb = ipool.tile([P, NT, SLEN], F32R, tag="xk")
            xv_sb = ipool.tile([P, NT, SLEN], F32R, tag="xv")
            for s in range(NT):
                nc.sync.dma_start(xk_sb[:, s], xk_d[:, s].bitcast(F32R))
            for s in range(NT):
                nc.sync.dma_start(xv_sb[:, s], xv_d[:, s].bitcast(F32R))
            nc.sync.dma_start(
                wp[:, QW_O : QW_O + 2048], wpack_d[:, QW_O : QW_O + 2048].bitcast(F32R)
            )
            xq_sb = ipool.tile([P, 2, FR], F32R, tag="xq")
            nc.sync.dma_start(xq_sb[:], xq_d.bitcast(F32R))

            # ---- K / V convs, channel-major [ch, m] ----
            k_sb = mpool.tile([P, 2, NLOC], F32R, tag="mid")
            v_sb = mpool.tile([P, 2, NLOC], F32R, tag="mid")

            def conv_cm(src_sb, w_v, bias_off, dst, flip):
                # src_sb [P, NT, 2, 16*W]; strip s == free tile s
                for occ in range(2):
                    for s in range(NT):
                        ps = psp.tile([P, TW], F32, tag="ps")
                        psv = ps[:].rearrange("p (i j) -> p i j", j=NJ)
                        first = True
                        for icc in range(2):
                            sv = src_sb[:, s, icc, :].rearrange(
                                "p (r w) -> p r w", w=W_IMG
                            )
                            for dd in range(4):
                                di, dj = dd // 2, dd % 2
                                nc.tensor.matmul(
                                    psv,
                                    w_v[:, icc, dd, occ * P : (occ + 1) * P],
                                    sv[:, di::2, dj::2],
                                    start=first,
                                    stop=(icc == 1 and dd == 3),
                                )
                                first = False
                        dslice = dst[:, occ, s * TW : (s + 1) * TW]
                        if flip:
                            nc.scalar.activation(
                                dslice, ps[:], IDENT,
                                bias=sca(bias_off)[:, occ : occ + 1], scale=1.0,
                            )
                        else:
                            nc.vector.tensor_tensor(
                                dslice, ps[:],
                                sca(bias_off)[:, occ : occ + 1].to_broadcast([P, TW]),
                                ADD,
                            )

            conv_cm(
                xk_sb[:].rearrange("p s (i f) -> p s i f", i=2), kw_v, KB_O, k_sb, False
            )
            conv_cm(
                xv_sb[:].rearrange("p s (i f) -> p s i f", i=2), vw_v, VB_O, v_sb, True
            )

            # ---- per-chunk transpose + Gram accumulation ----
            gps = [
                psp.tile([P, C], F32, tag="ps", name=f"gps{g}") for g in range(2)
            ]
            flip = 0
            for t in range(NMT):
                ktt = tpool.tile([P, C], F32R, tag="ktt")
                vtt = tpool.tile([P, C], F32R, tag="vtt")
                for src, dst in ((k_sb, ktt), (v_sb, vtt)):
                    for occ in range(2):
                        tp = psp.tile([P, P], F32R, tag="ps")
                        nc.tensor.transpose(
                            tp[:], src[:, occ, t * P : (t + 1) * P], id_v
                        )
                        dsl = dst[:, occ * P : (occ + 1) * P]
                        if flip % 2:
                            nc.scalar.copy(dsl, tp[:])
                        else:
                            nc.vector.tensor_copy(dsl, tp[:])
                        flip += 1
                for g in range(2):
                    nc.tensor.matmul(
                        gps[g][:],
                        vtt[:, g * P : (g + 1) * P],
                        ktt[:],
                        start=(t == 0),
                        stop=(t == NMT - 1),
                        skip_group_check=True,
                    )

            gstage = wpool.tile([P, 2, P], F32, tag="gstage")
            for g in range(2):
                nc.vector.tensor_scalar_mul(
                    gstage[:, g, :],
                    gps[g][:, g * P : (g + 1) * P],
                    sca(GSC_O)[:, g : g + 1],
                )
            nc.sync.dma_start(gpart_d, gstage[:])
            nc.gpsimd.collective_compute(
                "AllReduce",
                ADD,
                replica_groups=[[0, 1], [2, 3], [4, 5], [6, 7]],
                ins=[gpart_d],
                outs=[gred_d],
            )
            g_sb = wpool.tile([P, 2, P], F32R, tag="gsb")
            for g in range(2):
                for hh in range(2):
                    r0 = 64 * hh
                    r1 = 64 - r0
                    nc.sync.dma_start(
                        g_sb[r0 : r0 + 64, g, r0 : r0 + 64],
                        gred_d[
                            r0 : r0 + 64, g * P + r0 : g * P + r0 + 64
                        ].bitcast(F32R),
                    )
                    nc.sync.dma_start(
                        g_sb[r0 : r0 + 64, g, r1 : r1 + 64], zz_d.bitcast(F32R)
                    )

            # ---- Q conv (fills the collective's latency) ----
            q_sb = wpool.tile([P, 2, NLOC], F32R, tag="q")
            xqv = [
                xq_sb[:, icc, :].rearrange("p (r w) -> p r w", w=W_IMG)
                for icc in range(2)
            ]
            for qcc in range(2):
                for nt in range(NT):
                    ps = psp.tile([P, TW], F32, tag="ps")
                    psv = ps[:].rearrange("p (i j) -> p i j", j=NJ)
                    first = True
                    for icc in range(2):
                        for dd in range(4):
                            di, dj = dd // 2, dd % 2
                            nc.tensor.matmul(
                                psv,
                                qw_v[:, icc, dd, qcc * P : (qcc + 1) * P],
                                xqv[icc][:, 16 * nt + di : 16 * nt + 16 : 2, dj::2],
                                start=first,
                                stop=(icc == 1 and dd == 3),
                            )
                            first = False
                    if nt % 2:
                        nc.scalar.activation(
                            q_sb[:, qcc, nt * TW : (nt + 1) * TW], ps[:], IDENT,
                            bias=sca(QB_O)[:, qcc : qcc + 1], scale=1.0,
                        )
                    else:
                        nc.vector.tensor_tensor(
                            q_sb[:, qcc, nt * TW : (nt + 1) * TW], ps[:],
                            sca(QB_O)[:, qcc : qcc + 1].to_broadcast([P, TW]),
                            ADD,
                        )

            # ---- W_cat^T = blockdiag(G) @ fw2^T ----
            w_sb = wpool.tile([P, 2, C], F32R, tag="w")
            for g in range(2):
                psw = psp.tile([P, C], F32, tag="ps")
                nc.tensor.matmul(
                    psw[:], g_sb[:, g, :], fw2_v[:, g, :], start=True, stop=True
                )
                nc.vector.tensor_copy(w_sb[:, g, :], psw[:])

            # ---- z'' = W_cat^T.T @ Q + fb ----
            z_sb = wpool.tile([P, 2, NLOC], F32, tag="z")
            for oc in range(2):
                for nt in range(NT):
                    ps = psp.tile([P, TW], F32, tag="ps")
                    for g in range(2):
                        nc.tensor.matmul(
                            ps[:],
                            w_sb[:, g, oc * P : (oc + 1) * P],
                            q_sb[:, g, nt * TW : (nt + 1) * TW],
                            start=(g == 0),
                            stop=(g == 1),
                        )
                    if nt % 2:
                        nc.scalar.activation(
                            z_sb[:, oc, nt * TW : (nt + 1) * TW], ps[:], IDENT,
                            bias=sca(FB_O)[:, oc : oc + 1], scale=1.0,
                        )
                    else:
                        nc.vector.tensor_tensor(
                            z_sb[:, oc, nt * TW : (nt + 1) * TW], ps[:],
                            sca(FB_O)[:, oc : oc + 1].to_broadcast([P, TW]),
                            ADD,
                        )

            # ---- pose term + fused 2x upsample-add of z'' ----
            for oc in range(2):
                zv = z_sb[:, oc, :].rearrange("p (i j) -> p i j", j=NJ)
                for half in range(2):
                    ost = mpool.tile([P, 6 * TW], F32, tag="mid")
                    for k in range(6):
                        ot = half * 6 + k
                        ps = psp.tile([P, TW], F32, tag="ps")
                        for icc in range(2):
                            nc.tensor.matmul(
                                ps[:],
                                fw1_v[:, icc, oc * P : (oc + 1) * P],
                                xq_sb[:, icc, ot * TW : (ot + 1) * TW],
                                start=(icc == 0),
                                stop=(icc == 1),
                            )
                        psv = ps[:].rearrange(
                            "p (i ri j rj) -> p i ri j rj", i=2, ri=2, j=NJ, rj=2
                        )
                        stv = ost[:, k * TW : (k + 1) * TW].rearrange(
                            "p (i ri j rj) -> p i ri j rj", i=2, ri=2, j=NJ, rj=2
                        )
                        zb = zv[:, 2 * ot : 2 * ot + 2, :, None].to_broadcast(
                            [P, 2, NJ, 2]
                        )
                        for ri in range(2):
                            nc.vector.tensor_tensor(
                                stv[:, :, ri, :, :], psv[:, :, ri, :, :], zb, ADD
                            )
                    nc.sync.dma_start(
                        out_d[:, oc, half * 6 * TW : (half + 1) * 6 * TW], ost[:]
                    )

    nc.compile()
    return nc


def _prep_inputs(inputs):
    """Build the 8 per-core input maps (host-side shard + weight packing)."""
    f = np.float32
    qw, qb = np.asarray(inputs["qw"], f), np.asarray(inputs["qb"], f)
    kw, kb = np.asarray(inputs["kw"], f), np.asarray(inputs["kb"], f)
    vw, vb = np.asarray(inputs["vw"], f), np.asarray(inputs["vb"], f)
    gamma = np.asarray(inputs["gamma"], f)
    fw, fb = np.asarray(inputs["fw"], f), np.asarray(inputs["fb"], f)
    pose = np.asarray(inputs["pose_enc"], f)
    app_pose = np.asarray(inputs["app_pose_enc"], f)
    app = np.asarray(inputs["app_enc"], f)

    wpack = np.zeros((P, WLEN), dtype=f)

    def packw(dst_off, w):
        # w [oc, ic, 2, 2] -> [p, icc, dd, oc]
        t = w.transpose(1, 2, 3, 0).reshape(2, P, 4, C).transpose(1, 0, 2, 3)
        wpack[:, dst_off : dst_off + 2048] = t.reshape(P, 2048)

    packw(QW_O, qw)
    packw(KW_O, kw)
    packw(VW_O, vw)
    wpack[:, FW1_O : FW1_O + 512] = (
        fw[:, :C, 0, 0].T.reshape(2, P, C).transpose(1, 0, 2).reshape(P, 512)
    )
    wpack[:, FW2_O : FW2_O + 512] = (
        fw[:, C:, 0, 0].T.reshape(2, P, C).transpose(1, 0, 2).reshape(P, 512)
    )
    wpack[:, ID_O : ID_O + P] = np.eye(P, dtype=f)
    wpack[:, QB_O : QB_O + 2] = qb.reshape(2, P).T
    wpack[:, FB_O : FB_O + 2] = fb.reshape(2, P).T
    gsc = (np.repeat(gamma.astype(np.float64), 64) / 8.0).astype(f)
    wpack[:, GSC_O : GSC_O + 2] = gsc.reshape(2, P).T
    wpack[:, KB_O : KB_O + 2] = kb.reshape(2, P).T
    wpack[:, VB_O : VB_O + 2] = vb.reshape(2, P).T
    zz = np.zeros((64, 64), dtype=f)

    def shard_q(x, b, h):  # [p, icc, fr]
        halfimg = x[b, :, RH * h : RH * (h + 1), :].reshape(2, P, FR)
        return np.ascontiguousarray(halfimg.transpose(1, 0, 2))

    def shard_kv(x, b, h):  # [p, strip, icc*1536]
        halfimg = x[b, :, RH * h : RH * (h + 1), :].reshape(2, P, NT, SLEN // 2)
        return np.ascontiguousarray(halfimg.transpose(1, 2, 0, 3).reshape(P, NT, SLEN))

    in_maps = []
    for c in range(8):
        b, h = c // 2, c % 2
        in_maps.append({
            "xq": shard_q(pose, b, h),
            "xk": shard_kv(app_pose, b, h),
            "xv": shard_kv(app, b, h),
            "wpack": wpack,
            "zz": zz,
        })
    return in_maps


def _get_runner():
    global _CACHED_NC, _RUNNER
    if _CACHED_NC is None:
        _CACHED_NC = _build()
    if _RUNNER is None:
        _RUNNER = _make_runner(_CACHED_NC)
    return _RUNNER


def _assemble(results):
    out = np.empty((4, C, W_IMG, W_IMG), dtype=np.float32)
    for c in range(8):
        b, h = c // 2, c % 2
        o = results[c]["out"]  # [P, 2, FR]
        out[b, :, RH * h : RH * (h + 1), :] = o.transpose(1, 0, 2).reshape(
            C, RH, W_IMG
        )
    return out


def kernel(**inputs):
    run = _get_runner()
    in_maps = _prep_inputs(inputs)
    return _assemble(run(in_maps))
